# revision 10
# baseline (speedup 1.0000x reference)
"""ARMA GNN message-passing kernel for 8 Trainium2 NeuronCores.

Strategy (graph-partitioned, per sharding hint):
  - Nodes sharded contiguously: core i owns nodes [12500*i, 12500*(i+1)).
  - Edges sharded by destination core; per-core edges laid out in "slots"
    grouped by source block (4 blocks of 25088 padded-table rows) so that
    dma_gather int16 indices stay in range.
  - dinv[src] is folded into the gather tables (T0 = dinv*(x@init_w),
    T1 = dinv*(h@arma_w)); dinv[dst] is applied after the segment sum.
    Per-edge weight ew from the edge MLP scales gathered rows.
  - deg/prop segment sums via gpsimd indirect_dma_start(compute_op=add)
    into DRAM, 128 edges (one per partition) per call; edge slots are
    interleaved by rank-within-destination so a call never carries the
    same destination twice (the SDMA CCE read-modify-write is not atomic).
  - T0/T1 replicated across cores via AllGather collectives.
"""

import numpy as np

N_NODES = 100_000
NSH = 12_500          # nodes per core
NSHP = 12_544         # padded to multiple of 128
NT = NSHP // 128      # 98 node tiles per core
TBL = 8 * NSHP        # padded full-table rows (100352)
BLK = TBL // 4        # gather source block size (25088 < 32768)
CALL = 4096           # edges per gather/scatter call
F_IN, F_HID, F_OUT, E_DIM = 128, 64, 64, 16
N_CORES = 8


def _round_up(x, m):
    return (x + m - 1) // m * m


def _host_prep(x, edge_index, edge_attr, mlp_w1, mlp_b1, mlp_w2, mlp_b2,
               init_w, arma_w, root_w0, root_w1, b0, b1, lin_w, lin_b):
    row = edge_index[0].astype(np.int64)
    col = edge_index[1].astype(np.int64)
    core_of = col // NSH

    per_core = []
    for c in range(N_CORES):
        m = core_of == c
        r = row[m]
        d = (col[m] - c * NSH).astype(np.int64)
        ea = edge_attr[m]
        src_pad = (r // NSH) * NSHP + (r % NSH)   # row in padded table
        n = r.shape[0]
        # interleave by rank-within-destination so each 128-slot scatter call
        # sees distinct destination rows (CCE RMW adds are not atomic), and
        # pad every rank segment to a 128 boundary
        bydest = np.argsort(d, kind="stable")
        ds = d[bydest]
        newseg = np.flatnonzero(np.diff(ds) != 0) + 1
        seg_id = np.zeros(n, np.int64)
        seg_id[newseg] = 1
        seg_id = np.cumsum(seg_id)
        first_pos = np.full(seg_id.max() + 1, n, np.int64)
        np.minimum.at(first_pos, seg_id, np.arange(n))
        rank = np.arange(n) - first_pos[seg_id]
        order = bydest[np.lexsort((ds, rank))]
        counts_r = np.bincount(rank)
        pieces = []
        pos = 0
        for cnt in counts_r.tolist():
            pieces.append(order[pos:pos + cnt])
            pos += cnt
            pieces.append(np.full((-cnt) % 128, -1, np.int64))
        slots = np.concatenate(pieces)
        per_core.append((src_pad, d, ea, slots))

    e_pad = _round_up(max(len(pc[3]) for pc in per_core), 1024)
    n_calls = e_pad // 128
    call_blk = None

    # slot -> MLP position mapping (see kernel MLP loop):
    #   s = m + 128*cc ; cc = 8*t + 2*k + j ; u = 512*t + 128*k + m ; half g = j
    s = np.arange(e_pad)
    m_ = s % 128
    cc = s // 128
    j_ = cc % 2
    k_ = (cc // 2) % 4
    t_ = cc // 8
    u_ = 512 * t_ + 128 * k_ + m_

    in_maps = []
    w1d = np.zeros((32, 128), np.float32)
    w1d[:16, :64] = mlp_w1
    w1d[16:, 64:] = mlp_w1
    w2s = np.zeros((128, 2), np.float32)
    w2s[:64, 0] = mlp_w2[:, 0]
    w2s[64:, 1] = mlp_w2[:, 0]
    b1s = np.concatenate([mlp_b1, mlp_b1]).astype(np.float32)[:, None]
    common = {
        "w1d": w1d, "w2s": w2s, "b1s": b1s,
        "initw": init_w.astype(np.float32),
        "rw0": root_w0.astype(np.float32),
        "rw1": root_w1.astype(np.float32),
        "armaw": arma_w.astype(np.float32),
        "linw": lin_w.astype(np.float32),
        "b0c": b0.astype(np.float32)[:, None],
        "b1c": b1.astype(np.float32)[:, None],
        "linbc": lin_b.astype(np.float32)[:, None],
        "i64": np.eye(64, dtype=np.float32),
        "i128": np.eye(128, dtype=np.float32),
    }

    for c in range(N_CORES):
        src_pad, d, ea, slots = per_core[c]
        slots = np.concatenate([slots, np.full(e_pad - len(slots), -1, np.int64)])
        real = slots >= 0
        sl = np.where(real, slots, 0)
        gidx = np.where(real, src_pad[sl], 0).astype(np.int32)
        sidx = np.where(real, d[sl], NSHP - 1).astype(np.int32)   # pads -> trash row
        attr = np.where(real[:, None], ea[sl], 0.0).astype(np.float32)

        ea2 = np.zeros((32, e_pad // 2), np.float32)
        for g in (0, 1):
            sel = j_ == g
            ea2[16 * g:16 * (g + 1), u_[sel]] = attr[sel].T

        xs = np.zeros((NSHP, F_IN), np.float32)
        xs[:NSH] = x[c * NSH:(c + 1) * NSH]
        im = dict(common)
        im["xT"] = np.ascontiguousarray(xs.T)           # [128, NSHP]
        im["ea2"] = ea2                                  # [32, e_pad//2]
        im["gidx"] = np.ascontiguousarray(gidx.reshape(-1, 128).T)  # [128, C]
        im["sidx"] = np.ascontiguousarray(sidx.reshape(-1, 128).T)
        in_maps.append(im)

    return in_maps, e_pad, n_calls, call_blk, float(mlp_b2[0])


def _build_nc(e_pad, n_calls, call_blk, b2f):
    import concourse.bacc as bacc
    import concourse.bass as bass
    import concourse.mybir as mybir
    import concourse.tile as tile

    f32 = mybir.dt.float32
    i16 = mybir.dt.int16
    i32 = mybir.dt.int32
    AF = mybir.ActivationFunctionType
    OP = mybir.AluOpType

    C = e_pad // 128          # ew columns
    IW = e_pad // 128

    nc = bacc.Bacc("TRN2", num_devices=N_CORES)
    t_xT = nc.dram_tensor("xT", [128, NSHP], f32, kind="ExternalInput")
    t_ea2 = nc.dram_tensor("ea2", [32, e_pad // 2], f32, kind="ExternalInput")
    t_gidx = nc.dram_tensor("gidx", [128, IW], i32, kind="ExternalInput")
    t_sidx = nc.dram_tensor("sidx", [128, IW], i32, kind="ExternalInput")
    t_w1d = nc.dram_tensor("w1d", [32, 128], f32, kind="ExternalInput")
    t_w2s = nc.dram_tensor("w2s", [128, 2], f32, kind="ExternalInput")
    t_b1s = nc.dram_tensor("b1s", [128, 1], f32, kind="ExternalInput")
    t_initw = nc.dram_tensor("initw", [128, 64], f32, kind="ExternalInput")
    t_rw0 = nc.dram_tensor("rw0", [128, 64], f32, kind="ExternalInput")
    t_rw1 = nc.dram_tensor("rw1", [128, 64], f32, kind="ExternalInput")
    t_armaw = nc.dram_tensor("armaw", [64, 64], f32, kind="ExternalInput")
    t_linw = nc.dram_tensor("linw", [64, 64], f32, kind="ExternalInput")
    t_b0c = nc.dram_tensor("b0c", [64, 1], f32, kind="ExternalInput")
    t_b1c = nc.dram_tensor("b1c", [64, 1], f32, kind="ExternalInput")
    t_linbc = nc.dram_tensor("linbc", [64, 1], f32, kind="ExternalInput")
    t_i64 = nc.dram_tensor("i64", [64, 64], f32, kind="ExternalInput")
    t_i128 = nc.dram_tensor("i128", [128, 128], f32, kind="ExternalInput")
    t_out = nc.dram_tensor("out", [NSHP, F_OUT], f32, kind="ExternalOutput")

    with tile.TileContext(nc) as tc:
        with (
            tc.tile_pool(name="const", bufs=1) as cpool,
            tc.tile_pool(name="sb", bufs=3) as sb,
            tc.tile_pool(name="ea", bufs=3) as eapool,
            tc.tile_pool(name="h1", bufs=3) as h1pool,
            tc.tile_pool(name="val", bufs=4) as valpool,
            tc.tile_pool(name="nm", bufs=4) as nmpool,
            tc.tile_pool(name="dv", bufs=8) as dvpool,
            tc.tile_pool(name="ps_big", bufs=2, space="PSUM") as psb,
            tc.tile_pool(name="ps_ew", bufs=2, space="PSUM") as psew,
            tc.tile_pool(name="ps_sm", bufs=3, space="PSUM") as pss,
            tc.tile_pool(name="dram", bufs=1, space="DRAM") as dram,
        ):
            # ---- resident constants / indices ----
            w1d = cpool.tile([32, 128], f32)
            nc.sync.dma_start(w1d[:], t_w1d[:])
            w2s = cpool.tile([128, 2], f32)
            nc.sync.dma_start(w2s[:], t_w2s[:])
            b1s = cpool.tile([128, 1], f32)
            nc.sync.dma_start(b1s[:], t_b1s[:])
            initw = cpool.tile([128, 64], f32)
            nc.sync.dma_start(initw[:], t_initw[:])
            rw0 = cpool.tile([128, 64], f32)
            nc.sync.dma_start(rw0[:], t_rw0[:])
            rw1 = cpool.tile([128, 64], f32)
            nc.sync.dma_start(rw1[:], t_rw1[:])
            armaw = cpool.tile([64, 64], f32)
            nc.sync.dma_start(armaw[:], t_armaw[:])
            linw = cpool.tile([64, 64], f32)
            nc.sync.dma_start(linw[:], t_linw[:])
            b0c = cpool.tile([64, 1], f32)
            nc.sync.dma_start(b0c[:], t_b0c[:])
            b1c = cpool.tile([64, 1], f32)
            nc.sync.dma_start(b1c[:], t_b1c[:])
            linbc = cpool.tile([64, 1], f32)
            nc.sync.dma_start(linbc[:], t_linbc[:])
            i64 = cpool.tile([64, 64], f32)
            nc.sync.dma_start(i64[:], t_i64[:])
            i128 = cpool.tile([128, 128], f32)
            nc.sync.dma_start(i128[:], t_i128[:])
            gidx = cpool.tile([128, IW], i32)
            nc.sync.dma_start(gidx[:], t_gidx[:])
            sidx = cpool.tile([128, IW], i32)
            nc.sync.dma_start(sidx[:], t_sidx[:])
            ew = cpool.tile([128, C], f32)
            dinv = cpool.tile([128, NT], f32)

            deg_d = dram.tile([NSHP, 64], f32)
            prop0_d = dram.tile([NSHP, 64], f32)
            prop1_d = dram.tile([NSHP, 64], f32)
            t0_sh = dram.tile([NSHP, 64], f32)
            t1_sh = dram.tile([NSHP, 64], f32)
            t0_full = dram.tile([TBL, 64], f32)
            t1_full = dram.tile([TBL, 64], f32)

            # zero the scatter accumulators
            ztile = cpool.tile([128, NT * 64], f32)
            nc.vector.memset(ztile[:], 0.0)
            for dd in (deg_d, prop0_d, prop1_d):
                nc.sync.dma_start(
                    dd[:].rearrange("(t p) f -> p t f", p=128),
                    ztile[:].rearrange("p (t f) -> p t f", f=64),
                )

            # ---- edge MLP -> ew [128, C] ----
            n_grp = (C + 511) // 512
            for g in range(n_grp):
                gcols = min(512, C - 512 * g)
                ewp = psew.tile([128, 512], f32, tag="ewp")
                for t2 in range(gcols // 8):
                    t = g * 64 + t2
                    ea_t = eapool.tile([32, 512], f32, tag="ea")
                    nc.sync.dma_start(ea_t[:], t_ea2[:, 512 * t:512 * (t + 1)])
                    h1p = psb.tile([128, 512], f32, tag="h1p")
                    nc.tensor.matmul(h1p[:], lhsT=w1d[:], rhs=ea_t[:],
                                     start=True, stop=True)
                    h1s = h1pool.tile([128, 512], f32, tag="h1s")
                    nc.scalar.activation(h1s[:], h1p[:], AF.Relu, bias=b1s[:])
                    for k in range(4):
                        nc.tensor.matmul(
                            ewp[:, 8 * t2 + 2 * k: 8 * t2 + 2 * k + 2],
                            lhsT=h1s[:, 128 * k:128 * (k + 1)],
                            rhs=w2s[:],
                            start=True, stop=True,
                        )
                nc.scalar.activation(ew[:, 512 * g:512 * g + gcols],
                                     ewp[:, :gcols], AF.Relu, bias=b2f)

            # ---- degree scatter (128 edges per indirect call) ----
            ones_t = cpool.tile([128, 64], f32)
            nc.vector.memset(ones_t[:], 1.0)
            for ci in range(n_calls):
                vt = valpool.tile([128, 64], f32, tag="val")
                nc.vector.tensor_scalar(vt[:], ones_t[:], ew[:, ci:ci + 1],
                                        None, OP.mult)
                nc.gpsimd.indirect_dma_start(
                    out=deg_d[:],
                    out_offset=bass.IndirectOffsetOnAxis(
                        ap=sidx[:, ci:ci + 1], axis=0),
                    in_=vt[:], in_offset=None, compute_op=OP.add)

            # ---- dinv = where(deg>0, 1/sqrt(deg), 0) ----
            degc = dvpool.tile([128, NT], f32, tag="dv")
            nc.sync.dma_start(
                degc[:].rearrange("p (t o) -> p t o", o=1),
                deg_d[:].rearrange("(t p) f -> p t f", p=128)[:, :, 0:1],
            )
            mask = dvpool.tile([128, NT], f32, tag="dv")
            nc.vector.tensor_scalar(mask[:], degc[:], 0.0, None, OP.is_gt)
            nm = dvpool.tile([128, NT], f32, tag="dv")
            nc.vector.tensor_scalar(nm[:], mask[:], -1.0, 1.0, OP.mult, OP.add)
            safe = dvpool.tile([128, NT], f32, tag="dv")
            nc.vector.tensor_tensor(safe[:], degc[:], nm[:], OP.add)
            sq = dvpool.tile([128, NT], f32, tag="dv")
            nc.scalar.activation(sq[:], safe[:], AF.Sqrt)
            rec = dvpool.tile([128, NT], f32, tag="dv")
            nc.vector.reciprocal(rec[:], sq[:])
            r = rec
            for _ in range(2):   # Newton refine rsqrt: r <- r*(1.5 - 0.5*safe*r^2)
                r2 = dvpool.tile([128, NT], f32, tag="dv")
                nc.vector.tensor_tensor(r2[:], r[:], r[:], OP.mult)
                tchain = dvpool.tile([128, NT], f32, tag="dv")
                nc.vector.tensor_tensor(tchain[:], r2[:], safe[:], OP.mult)
                fch = dvpool.tile([128, NT], f32, tag="dv")
                nc.vector.tensor_scalar(fch[:], tchain[:], -0.5, 1.5, OP.mult, OP.add)
                rn = dvpool.tile([128, NT], f32, tag="dv")
                nc.vector.tensor_tensor(rn[:], r[:], fch[:], OP.mult)
                r = rn
            nc.vector.tensor_tensor(dinv[:], r[:], mask[:], OP.mult)

            # ---- helper: node-major scaled table tile from feat-major psum ----
            def to_table(lhs_sbuf_64x128, t, dst_dram):
                pst = pss.tile([128, 64], f32, tag="sm")
                nc.tensor.matmul(pst[:], lhsT=lhs_sbuf_64x128, rhs=i64[:],
                                 start=True, stop=True)
                tt = nmpool.tile([128, 64], f32, tag="tab")
                nc.vector.tensor_scalar(tt[:], pst[:], dinv[:, t:t + 1], None,
                                        OP.mult)
                nc.sync.dma_start(dst_dram[128 * t:128 * (t + 1), :], tt[:])

            # ---- T0 = dinv * (x @ init_w) ----
            for t in range(NT):
                xt = sb.tile([128, 128], f32, tag="xt")
                nc.sync.dma_start(xt[:], t_xT[:, 128 * t:128 * (t + 1)])
                p0 = pss.tile([64, 128], f32, tag="sm")
                nc.tensor.matmul(p0[:], lhsT=initw[:], rhs=xt[:],
                                 start=True, stop=True)
                s0 = nmpool.tile([64, 128], f32, tag="fmsb")
                nc.vector.tensor_copy(s0[:], p0[:])
                to_table(s0[:], t, t0_sh)

            nc.gpsimd.collective_compute(
                "AllGather", OP.bypass,
                replica_groups=[list(range(N_CORES))],
                ins=[t0_sh[:].opt()], outs=[t0_full[:].opt()],
            )

            # ---- propagate pass (shared for t=0 / t=1) ----
            def propagate(table_full, prop_dram):
                for ci in range(n_calls):
                    vt = valpool.tile([128, 64], f32, tag="val")
                    nc.gpsimd.indirect_dma_start(
                        out=vt[:], out_offset=None, in_=table_full[:],
                        in_offset=bass.IndirectOffsetOnAxis(
                            ap=gidx[:, ci:ci + 1], axis=0))
                    nc.vector.tensor_scalar(vt[:], vt[:], ew[:, ci:ci + 1],
                                            None, OP.mult)
                    nc.gpsimd.indirect_dma_start(
                        out=prop_dram[:],
                        out_offset=bass.IndirectOffsetOnAxis(
                            ap=sidx[:, ci:ci + 1], axis=0),
                        in_=vt[:], in_offset=None, compute_op=OP.add)

            propagate(t0_full, prop0_d)

            # ---- h = relu(dinv*prop0 + x@root_w0 + b0); T1 = dinv*(h@arma_w) ----
            for t in range(NT):
                pr = sb.tile([128, 64], f32, tag="pr")
                nc.sync.dma_start(pr[:], prop0_d[128 * t:128 * (t + 1), :])
                prs = sb.tile([128, 64], f32, tag="prs")
                nc.vector.tensor_scalar(prs[:], pr[:], dinv[:, t:t + 1], None,
                                        OP.mult)
                xt = sb.tile([128, 128], f32, tag="xt")
                nc.sync.dma_start(xt[:], t_xT[:, 128 * t:128 * (t + 1)])
                pc = pss.tile([64, 128], f32, tag="sm")
                nc.tensor.matmul(pc[:], lhsT=rw0[:], rhs=xt[:],
                                 start=True, stop=False)
                nc.tensor.matmul(pc[:], lhsT=prs[:], rhs=i128[:],
                                 start=False, stop=True)
                hT = nmpool.tile([64, 128], f32, tag="fmsb")
                nc.scalar.activation(hT[:], pc[:], AF.Relu, bias=b0c[:])
                pd = pss.tile([64, 128], f32, tag="sm")
                nc.tensor.matmul(pd[:], lhsT=armaw[:], rhs=hT[:],
                                 start=True, stop=True)
                sd = nmpool.tile([64, 128], f32, tag="fmsb2")
                nc.vector.tensor_copy(sd[:], pd[:])
                to_table(sd[:], t, t1_sh)

            nc.gpsimd.collective_compute(
                "AllGather", OP.bypass,
                replica_groups=[list(range(N_CORES))],
                ins=[t1_sh[:].opt()], outs=[t1_full[:].opt()],
            )

            propagate(t1_full, prop1_d)

            # ---- out = relu(dinv*prop1 + x@root_w1 + b1) @ lin_w + lin_b ----
            for t in range(NT):
                pr = sb.tile([128, 64], f32, tag="pr")
                nc.sync.dma_start(pr[:], prop1_d[128 * t:128 * (t + 1), :])
                prs = sb.tile([128, 64], f32, tag="prs")
                nc.vector.tensor_scalar(prs[:], pr[:], dinv[:, t:t + 1], None,
                                        OP.mult)
                xt = sb.tile([128, 128], f32, tag="xt")
                nc.sync.dma_start(xt[:], t_xT[:, 128 * t:128 * (t + 1)])
                pc = pss.tile([64, 128], f32, tag="sm")
                nc.tensor.matmul(pc[:], lhsT=rw1[:], rhs=xt[:],
                                 start=True, stop=False)
                nc.tensor.matmul(pc[:], lhsT=prs[:], rhs=i128[:],
                                 start=False, stop=True)
                rT = nmpool.tile([64, 128], f32, tag="fmsb")
                nc.scalar.activation(rT[:], pc[:], AF.Relu, bias=b1c[:])
                pg = pss.tile([64, 128], f32, tag="sm")
                nc.tensor.matmul(pg[:], lhsT=linw[:], rhs=rT[:],
                                 start=True, stop=True)
                og = nmpool.tile([64, 128], f32, tag="fmsb2")
                nc.vector.tensor_scalar(og[:], pg[:], linbc[:], None, OP.add)
                ph = pss.tile([128, 64], f32, tag="sm")
                nc.tensor.matmul(ph[:], lhsT=og[:], rhs=i64[:],
                                 start=True, stop=True)
                ot = nmpool.tile([128, 64], f32, tag="tab")
                nc.vector.tensor_copy(ot[:], ph[:])
                nc.sync.dma_start(t_out[128 * t:128 * (t + 1), :], ot[:])

    nc.compile()
    return nc


def kernel(**inputs):
    from concourse.bass_utils import run_bass_kernel_spmd

    inputs = {k: np.asarray(v) for k, v in inputs.items()}
    in_maps, e_pad, n_calls, call_blk, b2f = _host_prep(**inputs)
    nc = _build_nc(e_pad, n_calls, call_blk, b2f)
    res = run_bass_kernel_spmd(nc, in_maps, core_ids=list(range(N_CORES)))
    out = np.concatenate([r["out"][:NSH] for r in res.results], axis=0)
    return out.astype(np.float32)


if __name__ == "__main__":
    import reference
    ins = {k: np.asarray(v) for k, v in reference.setup_inputs().items()}
    got = kernel(**ins)
    exp = np.asarray(reference.reference(**ins))
    err = np.abs(got - exp).max() / (np.abs(exp).max() + 1e-30)
    print("Relative error:", err)
